# revision 3
# baseline (speedup 1.0000x reference)
"""Document-block-diagonal causal GQA attention on 8 trn2 NeuronCores.

Sharding: core i owns KV head i (tensor parallel over the 8 KV heads).
Each core computes its 4 GQA query heads x 4 docs = 16 independent
1024-token causal attentions with head_dim 128.

On-chip dataflow (per core, all layouts chosen so no on-chip transpose
is ever needed):
  - host feeds qT/kT pre-transposed to [d=128, tokens]
  - S^T blocks [k_part=128, q_free] = kT_chunk.T @ qT   (float32r matmuls)
  - P^T = exp(SCALE * S^T) on ScalarE, PSUM -> SBUF, cast to bf16
  - diagonal causal mask: bf16 multiply by 0/1 triangle on VectorE
  - O = P^T_chunk.T @ [V | 1] accumulated over k chunks in PSUM; the
    appended ones-column yields the softmax row-sums for free
  - normalize: reciprocal of the ones-column + tensor_scalar multiply,
    which doubles as the PSUM -> SBUF copy, then DMA out
"""

import math
import numpy as np
from contextlib import ExitStack

from concourse import bass, bacc, tile, mybir
from concourse.bass_utils import run_bass_kernel_spmd

FP32 = mybir.dt.float32
F32R = mybir.dt.float32r
BF16 = mybir.dt.bfloat16

NUM_HEADS = 32
NUM_KV_HEADS = 8
HEAD_DIM = 128
G = NUM_HEADS // NUM_KV_HEADS  # 4 query heads per KV head
S = 4096
NDOCS = 4
L = S // NDOCS  # 1024 tokens per doc
NSTRIP = L // 128  # 8 q/k strips of 128 per doc
NHD = G * NDOCS  # 16 (head, doc) pairs per core
SCALE = 1.0 / math.sqrt(HEAD_DIM)
N_CORES = 8

# q-chunk splits per k-strip kj: cover q in [128*kj, 1024). Each chunk
# is a separate matmul whose PSUM output must not cross a 512-element
# (2 KiB) bank boundary, so splits happen exactly at 512. float32r runs
# 1 cycle/row at N >= 256, 4 cycles/row below — only the kj=3 tail (128)
# and kj=7 (128) pay the narrow penalty.
def _chunks_for(e):
    if e <= 512:
        return [e]
    return [512, e - 512]


def _build_kernel_body(ctx, tc, qT, kT, vE, m01, out):
    nc = tc.nc

    qpool = ctx.enter_context(tc.tile_pool(name="qpool", bufs=3))
    cpool = ctx.enter_context(tc.tile_pool(name="cpool", bufs=1))
    ptpool = ctx.enter_context(tc.tile_pool(name="ptpool", bufs=16))
    opool = ctx.enter_context(tc.tile_pool(name="opool", bufs=2))
    spool = ctx.enter_context(tc.tile_pool(name="spool", bufs=8))
    psS_pool = ctx.enter_context(tc.tile_pool(name="psS", bufs=2, space="PSUM"))
    psO_pool = ctx.enter_context(tc.tile_pool(name="psO", bufs=2, space="PSUM"))

    # Whole-kernel resident tiles
    kT_sb = cpool.tile([128, NDOCS * L], F32R, tag="kT")
    nc.sync.dma_start(out=kT_sb[:], in_=kT[:])
    vE_sb = cpool.tile([128, NDOCS * NSTRIP, 129], BF16, tag="vE")
    nc.sync.dma_start(
        out=vE_sb[:], in_=vE.rearrange("(c p) d -> p c d", p=128)
    )
    m01_sb = cpool.tile([128, 128], BF16, tag="m01")
    nc.sync.dma_start(out=m01_sb[:], in_=m01[:])

    for hd in range(NHD):  # hd = h * NDOCS + n
        n = hd % NDOCS
        qT_sb = qpool.tile([128, L], F32R, tag="qT")
        nc.sync.dma_start(out=qT_sb[:], in_=qT[:, hd * L : (hd + 1) * L])

        # --- S^T blocks + exp -> P^T (bf16) ---
        pts = []
        for kj in range(NSTRIP):
            qoff = 128 * kj
            e = L - qoff
            psS = psS_pool.tile([128, L], FP32, tag="psS")
            off = 0
            for clen in _chunks_for(e):
                nc.tensor.matmul(
                    out=psS[:, off : off + clen],
                    lhsT=kT_sb[:, n * L + qoff : n * L + qoff + 128],
                    rhs=qT_sb[:, qoff + off : qoff + off + clen],
                    start=True,
                    stop=True,
                )
                off += clen
            pt = ptpool.tile([128, L], BF16, tag="pt")
            nc.scalar.activation(
                pt[:, qoff : qoff + e],
                psS[:, 0:e],
                mybir.ActivationFunctionType.Exp,
                scale=SCALE,
            )
            # causal mask inside the diagonal 128x128 block
            nc.vector.tensor_mul(
                pt[:, qoff : qoff + 128],
                pt[:, qoff : qoff + 128],
                m01_sb[:],
            )
            pts.append(pt)

        # --- PV + row-sum (ones column) + normalize ---
        o_sb = opool.tile([128, NSTRIP, 128], FP32, tag="o")
        for qi in range(NSTRIP):
            psO = psO_pool.tile([128, 129], FP32, tag="psO")
            for kj in range(qi + 1):
                nc.tensor.matmul(
                    out=psO[:],
                    lhsT=pts[kj][:, qi * 128 : qi * 128 + 128],
                    rhs=vE_sb[:, n * NSTRIP + kj, :],
                    start=(kj == 0),
                    stop=(kj == qi),
                )
            recip = spool.tile([128, 1], FP32, tag="recip")
            nc.vector.reciprocal(recip[:], psO[:, 128:129])
            nc.vector.tensor_scalar_mul(o_sb[:, qi, :], psO[:, 0:128], recip[:])
        nc.sync.dma_start(
            out=out[hd * L : (hd + 1) * L, :].rearrange("(qi p) d -> p qi d", p=128),
            in_=o_sb[:],
        )


_CACHED_NC = None


def _get_nc():
    global _CACHED_NC
    if _CACHED_NC is not None:
        return _CACHED_NC
    nc = bacc.Bacc("TRN2", target_bir_lowering=False, debug=False)
    qT = nc.dram_tensor("qT", [128, NHD * L], F32R, kind="ExternalInput").ap()
    kT = nc.dram_tensor("kT", [128, NDOCS * L], F32R, kind="ExternalInput").ap()
    vE = nc.dram_tensor("vE", [NDOCS * L, 129], BF16, kind="ExternalInput").ap()
    m01 = nc.dram_tensor("m01", [128, 128], BF16, kind="ExternalInput").ap()
    out = nc.dram_tensor("out", [NHD * L, 128], FP32, kind="ExternalOutput").ap()
    with tile.TileContext(nc) as tc:
        with ExitStack() as ctx:
            _build_kernel_body(ctx, tc, qT, kT, vE, m01, out)
    nc.compile()
    _CACHED_NC = nc
    return nc


def _prep_inputs(q, k, v):
    bf16_np = mybir.dt.np(BF16)
    q4 = np.asarray(q, np.float32).reshape(NDOCS, L, NUM_HEADS, HEAD_DIM)
    k4 = np.asarray(k, np.float32).reshape(NDOCS, L, NUM_KV_HEADS, HEAD_DIM)
    v2 = np.asarray(v, np.float32).reshape(S, NUM_KV_HEADS, HEAD_DIM)
    m01 = (np.arange(128)[None, :] >= np.arange(128)[:, None]).astype(bf16_np)
    in_maps = []
    for i in range(N_CORES):
        # [d, h, n, j] -> [128, (h*NDOCS + n)*L + j]
        qT = (
            q4[:, :, G * i : G * i + G, :]
            .transpose(3, 2, 0, 1)
            .reshape(128, NHD * L)
            .copy()
        )
        kT = k4[:, :, i, :].transpose(2, 0, 1).reshape(128, NDOCS * L).copy()
        vE = np.ones((S, 129), np.float32)
        vE[:, :128] = v2[:, i, :]
        in_maps.append(
            {
                "qT": qT,
                "kT": kT,
                "vE": vE.astype(bf16_np),
                "m01": m01,
            }
        )
    return in_maps


def _assemble(results):
    out_full = np.empty((1, NUM_HEADS, S, HEAD_DIM), np.float32)
    for i in range(N_CORES):
        oc = np.asarray(results[i]["out"], np.float32).reshape(G, S, HEAD_DIM)
        for h in range(G):
            out_full[0, G * i + h] = oc[h]
    return out_full


def kernel(q, k, v, cu_seqlens, _trace=False, _trace_kwargs=None):
    nc = _get_nc()
    in_maps = _prep_inputs(q, k, v)
    res = run_bass_kernel_spmd(
        nc,
        in_maps,
        list(range(N_CORES)),
        trace=_trace,
        **(_trace_kwargs or {}),
    )
    out_full = _assemble(res.results)
    if _trace:
        return out_full, res
    return out_full


# revision 5
# speedup vs baseline: 1.0234x; 1.0234x over previous
"""Document-block-diagonal causal GQA attention on 8 trn2 NeuronCores.

Sharding: core i owns KV head i (tensor parallel over the 8 KV heads).
Each core computes its 4 GQA query heads x 4 docs = 16 independent
1024-token causal attentions with head_dim 128.

On-chip dataflow (per core, all layouts chosen so no on-chip transpose
is ever needed):
  - host feeds qT/kT pre-transposed to [d=128, tokens]
  - S^T blocks [k_part=128, q_free] = kT_chunk.T @ qT   (float32r matmuls)
  - P^T = exp(SCALE * S^T) on ScalarE, PSUM -> SBUF, cast to bf16
  - diagonal causal mask: bf16 multiply by 0/1 triangle on VectorE
  - O = P^T_chunk.T @ [V | 1] accumulated over k chunks in PSUM; the
    appended ones-column yields the softmax row-sums for free
  - normalize: reciprocal of the ones-column + tensor_scalar multiply,
    which doubles as the PSUM -> SBUF copy, then DMA out
"""

import math
import numpy as np
from contextlib import ExitStack

from concourse import bass, bacc, tile, mybir
from concourse.bass_utils import run_bass_kernel_spmd

FP32 = mybir.dt.float32
F32R = mybir.dt.float32r
BF16 = mybir.dt.bfloat16

NUM_HEADS = 32
NUM_KV_HEADS = 8
HEAD_DIM = 128
G = NUM_HEADS // NUM_KV_HEADS  # 4 query heads per KV head
S = 4096
NDOCS = 4
L = S // NDOCS  # 1024 tokens per doc
NSTRIP = L // 128  # 8 q/k strips of 128 per doc
NHD = G * NDOCS  # 16 (head, doc) pairs per core
SCALE = 1.0 / math.sqrt(HEAD_DIM)
N_CORES = 8

# q-chunk splits per k-strip kj: cover q in [128*kj, 1024). Each chunk
# is a separate matmul whose PSUM output must not cross a 512-element
# (2 KiB) bank boundary, so splits happen exactly at 512. float32r runs
# 1 cycle/row at N >= 256, 4 cycles/row below — only the kj=3 tail (128)
# and kj=7 (128) pay the narrow penalty.
def _chunks_for(e):
    if e <= 512:
        return [e]
    return [512, e - 512]


def _build_kernel_body(ctx, tc, qT, kT, vE, m01, out):
    nc = tc.nc

    qpool = ctx.enter_context(tc.tile_pool(name="qpool", bufs=3))
    cpool = ctx.enter_context(tc.tile_pool(name="cpool", bufs=1))
    ptpool = ctx.enter_context(tc.tile_pool(name="ptpool", bufs=16))
    opool = ctx.enter_context(tc.tile_pool(name="opool", bufs=2))
    spool = ctx.enter_context(tc.tile_pool(name="spool", bufs=8))
    psS_pool = ctx.enter_context(tc.tile_pool(name="psS", bufs=3, space="PSUM"))
    psO_pool = ctx.enter_context(tc.tile_pool(name="psO", bufs=2, space="PSUM"))

    # Whole-kernel resident tiles (split per doc so doc-0 compute can
    # start before the rest of K/V arrives)
    m01_sb = cpool.tile([128, 128], BF16, tag="m01")
    nc.sync.dma_start(out=m01_sb[:], in_=m01[:])
    kT_sb = cpool.tile([128, NDOCS * L], F32R, tag="kT")
    vE_sb = cpool.tile([128, NDOCS * NSTRIP, 129], BF16, tag="vE")
    vEv = vE.rearrange("p (c d) -> p c d", d=129)
    for n in range(NDOCS):
        nc.sync.dma_start(
            out=kT_sb[:, n * L : (n + 1) * L], in_=kT[:, n * L : (n + 1) * L]
        )
        nc.sync.dma_start(
            out=vE_sb[:, n * NSTRIP : (n + 1) * NSTRIP, :],
            in_=vEv[:, n * NSTRIP : (n + 1) * NSTRIP, :],
        )

    for hd in range(NHD):  # hd = h * NDOCS + n
        n = hd % NDOCS
        qT_sb = qpool.tile([128, L], F32R, tag="qT")
        nc.sync.dma_start(out=qT_sb[:], in_=qT[:, hd * L : (hd + 1) * L])

        # --- S^T blocks + exp -> P^T (bf16) ---
        pts = []
        for kj in range(NSTRIP):
            qoff = 128 * kj
            e = L - qoff
            psS = psS_pool.tile([128, L], FP32, tag="psS")
            off = 0
            for clen in _chunks_for(e):
                nc.tensor.matmul(
                    out=psS[:, off : off + clen],
                    lhsT=kT_sb[:, n * L + qoff : n * L + qoff + 128],
                    rhs=qT_sb[:, qoff + off : qoff + off + clen],
                    start=True,
                    stop=True,
                )
                off += clen
            pt = ptpool.tile([128, L], BF16, tag="pt")
            nc.scalar.activation(
                pt[:, qoff : qoff + e],
                psS[:, 0:e],
                mybir.ActivationFunctionType.Exp,
                scale=SCALE,
            )
            # causal mask inside the diagonal 128x128 block
            nc.vector.tensor_mul(
                pt[:, qoff : qoff + 128],
                pt[:, qoff : qoff + 128],
                m01_sb[:],
            )
            pts.append(pt)

        # --- PV + row-sum (ones column) + normalize ---
        o_sb = opool.tile([128, NSTRIP, 128], FP32, tag="o")
        for qi in range(NSTRIP):
            psO = psO_pool.tile([128, 129], FP32, tag="psO")
            for kj in range(qi + 1):
                nc.tensor.matmul(
                    out=psO[:],
                    lhsT=pts[kj][:, qi * 128 : qi * 128 + 128],
                    rhs=vE_sb[:, n * NSTRIP + kj, :],
                    start=(kj == 0),
                    stop=(kj == qi),
                )
            recip = spool.tile([128, 1], FP32, tag="recip")
            nc.vector.reciprocal(recip[:], psO[:, 128:129])
            nc.vector.tensor_scalar_mul(o_sb[:, qi, :], psO[:, 0:128], recip[:])
        nc.sync.dma_start(
            out=out[:, hd * L : (hd + 1) * L], in_=o_sb[:].rearrange("p a b -> p (a b)")
        )


_CACHED_NC = None


def _get_nc():
    global _CACHED_NC
    if _CACHED_NC is not None:
        return _CACHED_NC
    nc = bacc.Bacc("TRN2", target_bir_lowering=False, debug=False)
    qT = nc.dram_tensor("qT", [128, NHD * L], F32R, kind="ExternalInput").ap()
    kT = nc.dram_tensor("kT", [128, NDOCS * L], F32R, kind="ExternalInput").ap()
    vE = nc.dram_tensor("vE", [128, NDOCS * NSTRIP * 129], BF16, kind="ExternalInput").ap()
    m01 = nc.dram_tensor("m01", [128, 128], BF16, kind="ExternalInput").ap()
    out = nc.dram_tensor("out", [128, NHD * L], FP32, kind="ExternalOutput").ap()
    with tile.TileContext(nc) as tc:
        with ExitStack() as ctx:
            _build_kernel_body(ctx, tc, qT, kT, vE, m01, out)
    nc.compile()
    _CACHED_NC = nc
    return nc


def _prep_inputs(q, k, v):
    bf16_np = mybir.dt.np(BF16)
    q4 = np.asarray(q, np.float32).reshape(NDOCS, L, NUM_HEADS, HEAD_DIM)
    k4 = np.asarray(k, np.float32).reshape(NDOCS, L, NUM_KV_HEADS, HEAD_DIM)
    v2 = np.asarray(v, np.float32).reshape(S, NUM_KV_HEADS, HEAD_DIM)
    m01 = (np.arange(128)[None, :] >= np.arange(128)[:, None]).astype(bf16_np)
    in_maps = []
    for i in range(N_CORES):
        # [d, h, n, j] -> [128, (h*NDOCS + n)*L + j]
        qT = (
            q4[:, :, G * i : G * i + G, :]
            .transpose(3, 2, 0, 1)
            .reshape(128, NHD * L)
            .copy()
        )
        kT = k4[:, :, i, :].transpose(2, 0, 1).reshape(128, NDOCS * L).copy()
        vE = np.ones((S, 129), np.float32)
        vE[:, :128] = v2[:, i, :]
        vE = (
            vE.reshape(NDOCS * NSTRIP, 128, 129)
            .transpose(1, 0, 2)
            .reshape(128, NDOCS * NSTRIP * 129)
        )
        in_maps.append(
            {
                "qT": qT,
                "kT": kT,
                "vE": vE.astype(bf16_np),
                "m01": m01,
            }
        )
    return in_maps


def _assemble(results):
    out_full = np.empty((1, NUM_HEADS, S, HEAD_DIM), np.float32)
    for i in range(N_CORES):
        oc = np.asarray(results[i]["out"], np.float32).reshape(
            128, G, NDOCS, NSTRIP, HEAD_DIM
        )
        # [p, h, n, qi, d] -> [h, (n, qi, p), d]
        oc = oc.transpose(1, 2, 3, 0, 4).reshape(G, S, HEAD_DIM)
        for h in range(G):
            out_full[0, G * i + h] = oc[h]
    return out_full


def kernel(q, k, v, cu_seqlens, _trace=False, _trace_kwargs=None):
    nc = _get_nc()
    in_maps = _prep_inputs(q, k, v)
    res = run_bass_kernel_spmd(
        nc,
        in_maps,
        list(range(N_CORES)),
        trace=_trace,
        **(_trace_kwargs or {}),
    )
    out_full = _assemble(res.results)
    if _trace:
        return out_full, res
    return out_full


# revision 7
# speedup vs baseline: 1.1333x; 1.1074x over previous
"""Document-block-diagonal causal GQA attention on 8 trn2 NeuronCores.

Sharding: core i owns KV head i (tensor parallel over the 8 KV heads).
Each core computes its 4 GQA query heads x 4 docs = 16 independent
1024-token causal attentions with head_dim 128.

On-chip dataflow (per core, all layouts chosen so no on-chip transpose
is ever needed):
  - host feeds qT/kT pre-transposed to [d=128, tokens]
  - S^T blocks [k_part=128, q_free] = kT_chunk.T @ qT   (float32r matmuls)
  - P^T = exp(SCALE * S^T) on ScalarE, PSUM -> SBUF, cast to bf16
  - diagonal causal mask: bf16 multiply by 0/1 triangle on VectorE
  - O = P^T_chunk.T @ [V | 1] accumulated over k chunks in PSUM; the
    appended ones-column yields the softmax row-sums for free
  - normalize: reciprocal of the ones-column + tensor_scalar multiply,
    which doubles as the PSUM -> SBUF copy, then DMA out
"""

import math
import numpy as np
from contextlib import ExitStack

from concourse import bass, bacc, tile, mybir
from concourse.bass_utils import run_bass_kernel_spmd

FP32 = mybir.dt.float32
F32R = mybir.dt.float32r
BF16 = mybir.dt.bfloat16

NUM_HEADS = 32
NUM_KV_HEADS = 8
HEAD_DIM = 128
G = NUM_HEADS // NUM_KV_HEADS  # 4 query heads per KV head
S = 4096
NDOCS = 4
L = S // NDOCS  # 1024 tokens per doc
NSTRIP = L // 128  # 8 q/k strips of 128 per doc
NHD = G * NDOCS  # 16 (head, doc) pairs per core
SCALE = 1.0 / math.sqrt(HEAD_DIM)
N_CORES = 8

# q-chunk splits per k-strip kj: cover q in [128*kj, 1024). Each chunk
# is a separate matmul whose PSUM output must not cross a 512-element
# (2 KiB) bank boundary, so splits happen exactly at 512. float32r runs
# 1 cycle/row at N >= 256, 4 cycles/row below — only the kj=3 tail (128)
# and kj=7 (128) pay the narrow penalty.
def _chunks_for(e):
    if e <= 512:
        return [e]
    return [512, e - 512]


def _build_kernel_body(ctx, tc, qT, kT, vE, m01, out):
    nc = tc.nc

    qpool = ctx.enter_context(tc.tile_pool(name="qpool", bufs=3))
    cpool = ctx.enter_context(tc.tile_pool(name="cpool", bufs=1))
    ptpool = ctx.enter_context(tc.tile_pool(name="ptpool", bufs=16))
    opool = ctx.enter_context(tc.tile_pool(name="opool", bufs=2))
    spool = ctx.enter_context(tc.tile_pool(name="spool", bufs=8))
    psS_pool = ctx.enter_context(tc.tile_pool(name="psS", bufs=3, space="PSUM"))
    psO_pool = ctx.enter_context(tc.tile_pool(name="psO", bufs=2, space="PSUM"))

    # Whole-kernel resident tiles (split per doc so doc-0 compute can
    # start before the rest of K/V arrives)
    m01_sb = cpool.tile([128, 128], BF16, tag="m01")
    nc.sync.dma_start(out=m01_sb[:], in_=m01[:])
    kT_sb = cpool.tile([128, NDOCS * L], F32R, tag="kT")
    vE_sb = cpool.tile([128, NDOCS * NSTRIP, 129], BF16, tag="vE")
    vEv = vE.rearrange("p (c d) -> p c d", d=129)
    for n in range(NDOCS):
        nc.sync.dma_start(
            out=kT_sb[:, n * L : (n + 1) * L], in_=kT[:, n * L : (n + 1) * L]
        )
        nc.sync.dma_start(
            out=vE_sb[:, n * NSTRIP : (n + 1) * NSTRIP, :],
            in_=vEv[:, n * NSTRIP : (n + 1) * NSTRIP, :],
        )

    # Software pipeline, one hd deep: round j of iteration hd emits the
    # QK+exp for (hd, kj=j) and the PV+normalize for (hd-1, qi=j), so
    # ScalarE exp of hd overlaps TensorE PV of hd-1 and no engine goes
    # idle between phases.
    def emit_qk_block(hd, kj, qT_sb):
        n = hd % NDOCS
        qoff = 128 * kj
        e = L - qoff
        psS = psS_pool.tile([128, L], FP32, tag="psS")
        off = 0
        for clen in _chunks_for(e):
            nc.tensor.matmul(
                out=psS[:, off : off + clen],
                lhsT=kT_sb[:, n * L + qoff : n * L + qoff + 128],
                rhs=qT_sb[:, qoff + off : qoff + off + clen],
                start=True,
                stop=True,
            )
            off += clen
        pt = ptpool.tile([128, L], BF16, tag="pt")
        nc.scalar.activation(
            pt[:, qoff : qoff + e],
            psS[:, 0:e],
            mybir.ActivationFunctionType.Exp,
            scale=SCALE,
        )
        # causal mask inside the diagonal 128x128 block
        nc.vector.tensor_mul(
            pt[:, qoff : qoff + 128], pt[:, qoff : qoff + 128], m01_sb[:]
        )
        return pt

    def emit_pv_strip(hd, qi, pts, o_sb):
        n = hd % NDOCS
        psO = psO_pool.tile([128, 129], FP32, tag="psO")
        for kj in range(qi + 1):
            nc.tensor.matmul(
                out=psO[:],
                lhsT=pts[kj][:, qi * 128 : qi * 128 + 128],
                rhs=vE_sb[:, n * NSTRIP + kj, :],
                start=(kj == 0),
                stop=(kj == qi),
            )
        recip = spool.tile([128, 1], FP32, tag="recip")
        nc.vector.reciprocal(recip[:], psO[:, 128:129])
        nc.vector.tensor_scalar_mul(o_sb[:, qi, :], psO[:, 0:128], recip[:])

    qts = {}
    prev_pts = None
    prev_o = None
    for hd in range(NHD + 1):
        if hd < NHD:
            qT_sb = qpool.tile([128, L], F32R, tag="qT")
            nc.sync.dma_start(out=qT_sb[:], in_=qT[:, hd * L : (hd + 1) * L])
            qts[hd] = qT_sb
        cur_pts = [] if hd < NHD else None
        cur_o = None
        if hd >= 1:
            cur_o = opool.tile([128, NSTRIP, 128], FP32, tag="o", name=f"o_{hd}")
        for j in range(NSTRIP):
            if hd < NHD:
                cur_pts.append(emit_qk_block(hd, j, qts[hd]))
            if hd >= 1:
                emit_pv_strip(hd - 1, j, prev_pts, cur_o)
        if hd >= 1:
            nc.sync.dma_start(
                out=out[:, (hd - 1) * L : hd * L],
                in_=cur_o[:].rearrange("p a b -> p (a b)"),
            )
            qts.pop(hd - 1, None)
        prev_pts = cur_pts
        prev_o = cur_o


_CACHED_NC = None


def _get_nc():
    global _CACHED_NC
    if _CACHED_NC is not None:
        return _CACHED_NC
    nc = bacc.Bacc("TRN2", target_bir_lowering=False, debug=False)
    qT = nc.dram_tensor("qT", [128, NHD * L], F32R, kind="ExternalInput").ap()
    kT = nc.dram_tensor("kT", [128, NDOCS * L], F32R, kind="ExternalInput").ap()
    vE = nc.dram_tensor("vE", [128, NDOCS * NSTRIP * 129], BF16, kind="ExternalInput").ap()
    m01 = nc.dram_tensor("m01", [128, 128], BF16, kind="ExternalInput").ap()
    out = nc.dram_tensor("out", [128, NHD * L], FP32, kind="ExternalOutput").ap()
    with tile.TileContext(nc) as tc:
        with ExitStack() as ctx:
            _build_kernel_body(ctx, tc, qT, kT, vE, m01, out)
    nc.compile()
    _CACHED_NC = nc
    return nc


def _prep_inputs(q, k, v):
    bf16_np = mybir.dt.np(BF16)
    q4 = np.asarray(q, np.float32).reshape(NDOCS, L, NUM_HEADS, HEAD_DIM)
    k4 = np.asarray(k, np.float32).reshape(NDOCS, L, NUM_KV_HEADS, HEAD_DIM)
    v2 = np.asarray(v, np.float32).reshape(S, NUM_KV_HEADS, HEAD_DIM)
    m01 = (np.arange(128)[None, :] >= np.arange(128)[:, None]).astype(bf16_np)
    in_maps = []
    for i in range(N_CORES):
        # [d, h, n, j] -> [128, (h*NDOCS + n)*L + j]
        qT = (
            q4[:, :, G * i : G * i + G, :]
            .transpose(3, 2, 0, 1)
            .reshape(128, NHD * L)
            .copy()
        )
        kT = k4[:, :, i, :].transpose(2, 0, 1).reshape(128, NDOCS * L).copy()
        vE = np.ones((S, 129), np.float32)
        vE[:, :128] = v2[:, i, :]
        vE = (
            vE.reshape(NDOCS * NSTRIP, 128, 129)
            .transpose(1, 0, 2)
            .reshape(128, NDOCS * NSTRIP * 129)
        )
        in_maps.append(
            {
                "qT": qT,
                "kT": kT,
                "vE": vE.astype(bf16_np),
                "m01": m01,
            }
        )
    return in_maps


def _assemble(results):
    out_full = np.empty((1, NUM_HEADS, S, HEAD_DIM), np.float32)
    for i in range(N_CORES):
        oc = np.asarray(results[i]["out"], np.float32).reshape(
            128, G, NDOCS, NSTRIP, HEAD_DIM
        )
        # [p, h, n, qi, d] -> [h, (n, qi, p), d]
        oc = oc.transpose(1, 2, 3, 0, 4).reshape(G, S, HEAD_DIM)
        for h in range(G):
            out_full[0, G * i + h] = oc[h]
    return out_full


def kernel(q, k, v, cu_seqlens, _trace=False, _trace_kwargs=None):
    nc = _get_nc()
    in_maps = _prep_inputs(q, k, v)
    res = run_bass_kernel_spmd(
        nc,
        in_maps,
        list(range(N_CORES)),
        trace=_trace,
        **(_trace_kwargs or {}),
    )
    out_full = _assemble(res.results)
    if _trace:
        return out_full, res
    return out_full
